# revision 28
# baseline (speedup 1.0000x reference)
"""Trainium2 Bass kernel for nn_CliffordJEPAModel.

Model = two GRU encoders (ctx / tgt) + tiny closed-form head.

Sharding: 8 cores = 2 encoders x 4 batch-quarters (B_local=16), no
cross-core communication.  The head (fc -> spectral norm -> closed-form
energy descent) runs on host in float64.

Device program (identical on all 8 cores, only data differs):
  * The recurrent matmul gh^T = Whh' @ h^T is LDWEIGHTS-bound on the PE
    array (108 weight tiles of 128x128 per step, ~27ns each in bf16).
    Whh is stored in fp8e4 (e4m3) to double the weight-ingest rate.
    Numerics: Whh is pre-scaled by S=8192 (power of two) on host so its
    values occupy e4m3's normal range; gi / biases are pre-scaled by S
    too, and the sigmoid/tanh activations descale via their input scale
    (= 1/S).  Everything after the activations is unscaled.  Validated
    in numpy: latent rel-err ~7e-3 (budget 2e-2).
  * Gate math is split into two UNEVEN psum groups (A = d-chunks 0-3,
    B = chunks 4-5).  Group A's psum completes at ~2/3 of the burst, so
    its DVE/ACT gate chain overlaps the rest of the burst instead of
    serializing after it (the v1 kernel lost ~2.8us/step to that).
  * The input matmul gi^T = Wih' @ X^T (stream-bound, N=512) is chopped
    into small quanta interleaved at the top of each recurrence step, so
    the phase-1 work rides in the h-latency slack of the step pipeline
    instead of running as a serial prefix.
"""

import os
import sys

for _p in ("/opt/trn_rl_repo/concourse", "/opt/trn_rl_repo"):
    if _p not in sys.path:
        sys.path.insert(0, _p)

import numpy as np
import ml_dtypes

import concourse.bacc as bacc
import concourse.mybir as mybir
import concourse.tile as tile
from concourse.bass_utils import run_bass_kernel_spmd

BF16 = ml_dtypes.bfloat16
FP8E4 = ml_dtypes.float8_e4m3  # concourse float8e4 <-> ml_dtypes.float8_e4m3 (max 240)

V, D, NB = 32000, 768, 8
B, S = 64, 256
DT_STEP, STEPS_DESC, PI = 0.1, 5, 3

N_CORES = 8
BQ = B // 4          # batch rows per core (16)
KT = D // 128        # 6 k-tiles
MT = 3 * KT          # 18 m-tiles of gate rows
NT = BQ * S          # tokens per core (4096)
CHT = 512            # tokens per gather/input-matmul chunk
NCH = NT // CHT      # 8 chunks
BLK = 16             # recurrence steps per gi prefetch block
HEAD_CH = 2          # chunks produced before the step loop starts
IN_Q = 5             # interleaved input-phase quanta per step

# Whh quantization mode: 'fp8' (e4m3 + S scaling) or 'bf16'.  Measured:
# LDWEIGHTS is row-rate-limited (27ns for 128 rows regardless of dtype),
# so fp8 weights buy nothing -- default to bf16 for accuracy.
MODE = os.environ.get("KMODE", "bf16")
SCALE = 4096.0 if MODE == "fp8" else 1.0  # max |Whh|*S ~ 148 < 240 (e4m3 max)

# equal gate groups: A = chunks 0..2, B = chunks 3..5.  A's chain runs on
# DVE while B's burst occupies the PE; B's chain runs on GpSimd.
GROUPS = ((0, 3), (3, 6))

F32 = mybir.dt.float32
BF16_T = mybir.dt.bfloat16
FP8_T = mybir.dt.float8e4
I16 = mybir.dt.int16
AF = mybir.ActivationFunctionType

WHH_T = FP8_T if MODE == "fp8" else BF16_T

# gate-row permutation: m-tile j = (chunk c=j//3, gate g=j%3) covers rows
# g*768 + c*128 .. +128  ->  interleaved [r_c, z_c, n_c] blocks.
_PERM = np.concatenate(
    [np.arange(g * D + c * 128, g * D + (c + 1) * 128) for c in range(KT) for g in range(3)]
)


def _build_program(steps=S):
    nc = bacc.Bacc("TRN2", target_bir_lowering=False, debug=False, num_devices=N_CORES)

    t_ident = nc.dram_tensor("ident", [128, 128], BF16_T, kind="ExternalInput")
    t_idx = nc.dram_tensor("idx", [128, NT // 16], I16, kind="ExternalInput")
    t_emb = nc.dram_tensor("emb", [V, D], BF16_T, kind="ExternalInput")
    t_wih = nc.dram_tensor("wihT", [128, KT * 3 * D], BF16_T, kind="ExternalInput")
    t_whh = nc.dram_tensor("whhT", [128, KT * 3 * D], WHH_T, kind="ExternalInput")
    t_bi = nc.dram_tensor("bias_i", [128, MT], F32, kind="ExternalInput")
    t_bn = nc.dram_tensor("bhhn", [128, KT * BQ], F32, kind="ExternalInput")
    t_out = nc.dram_tensor("h_out", [128, KT * BQ], F32, kind="ExternalOutput")

    W3D = 3 * D  # 2304
    n_blocks = steps // BLK
    n_chunks = max(1, (steps * BQ) // CHT)
    inv_s = 1.0 / SCALE

    with tile.TileContext(nc) as tc:
        with (
            tc.tile_pool(name="const", bufs=1) as const_pool,
            tc.tile_pool(name="dram", bufs=1, space="DRAM") as dram_pool,
            tc.tile_pool(name="xt", bufs=2) as xt_pool,
            tc.tile_pool(name="psum_in", bufs=2, space="PSUM") as psum_in,
            tc.tile_pool(name="gis", bufs=3) as gis_pool,
            tc.tile_pool(name="psA", bufs=2, space="PSUM") as psA_pool,
            tc.tile_pool(name="psB", bufs=2, space="PSUM") as psB_pool,
            tc.tile_pool(name="giblk", bufs=2) as giblk_pool,
            tc.tile_pool(name="hstate", bufs=1) as h_pool,
            tc.tile_pool(name="tmp", bufs=3) as tmp,
        ):
            ident_t = const_pool.tile([128, 128], BF16_T)
            nc.sync.dma_start(ident_t[:], t_ident.ap())
            idx_t = const_pool.tile([128, NT // 16], I16)
            wih_t = const_pool.tile([128, KT * W3D], BF16_T)
            whh_t = const_pool.tile([128, KT * W3D], WHH_T)
            bi_t = const_pool.tile([128, MT], F32)
            bn_t = const_pool.tile([128, KT * BQ], F32)
            nc.sync.dma_start(idx_t[:], t_idx.ap())
            nc.sync.dma_start(wih_t[:], t_wih.ap())
            nc.sync.dma_start(whh_t[:], t_whh.ap())
            nc.sync.dma_start(bi_t[:], t_bi.ap())
            nc.sync.dma_start(bn_t[:], t_bn.ap())

            giD = dram_pool.tile([MT, 128, NT], BF16_T)

            # ---- scheduler ordering helpers ----------------------------
            prev_pe_last = None   # last PE instr of previous segment

            def seg_edge(first_mm, last_mm):
                nonlocal prev_pe_last
                if first_mm is None:
                    return
                if prev_pe_last is not None:
                    tile.add_dep_helper(first_mm.ins, prev_pe_last.ins,
                                        sync=False, reason="pe segment order")
                prev_pe_last = last_mm

            act_prev = None

            def act_edge(op):
                nonlocal act_prev
                if act_prev is not None:
                    tile.add_dep_helper(op.ins, act_prev.ins, sync=False,
                                        reason="act order")
                act_prev = op

            # ---- input phase as a flat list of closures -----------------
            # each closure emits a tiny slice of (gather | matmul | act+store)
            # work and returns the matmul instruction if it emitted one.
            xt_tiles = {}
            ps_in_tiles = {}

            def mk_gather(nch):
                def f():
                    xt = xt_pool.tile([128, KT, CHT], BF16_T, name=f"xt{nch % 2}")
                    xt_tiles[nch] = xt
                    nc.gpsimd.dma_gather(
                        xt[:, :, :],
                        t_emb.ap(),
                        idx_t[:, nch * (CHT // 16):(nch + 1) * (CHT // 16)],
                        num_idxs=CHT,
                        num_idxs_reg=CHT,
                        elem_size=D,
                        transpose=True,
                    )
                    return None
                return f

            def mk_mm(nch, m, k):
                def f():
                    if k == 0:
                        ps_in_tiles[nch] = ps_in_tiles.get(nch, {})
                        ps_in_tiles[nch][m] = psum_in.tile([128, CHT], F32, name="ps_in")
                    mm = nc.tensor.matmul(
                        ps_in_tiles[nch][m][:],
                        wih_t[:, k * W3D + m * 128:k * W3D + (m + 1) * 128],
                        xt_tiles[nch][:, k, :],
                        start=(k == 0),
                        stop=(k == KT - 1),
                    )
                    return mm
                return f

            def mk_store(nch, m):
                def f():
                    gs = gis_pool.tile([128, CHT], BF16_T)
                    act = nc.scalar.activation(gs[:], ps_in_tiles[nch][m][:], AF.Identity,
                                               bias=bi_t[:, m:m + 1], scale=1.0)
                    act_edge(act)
                    nc.sync.dma_start(giD[m, :, nch * CHT:(nch + 1) * CHT], gs[:])
                    del ps_in_tiles[nch][m]
                    return None
                return f

            input_ops = []
            for nch in range(n_chunks):
                input_ops.append(mk_gather(nch))
                for m in range(MT):
                    for k in range(KT):
                        input_ops.append(mk_mm(nch, m, k))
                    input_ops.append(mk_store(nch, m))

            def pop_input(n):
                """Emit up to n input closures; return (first_mm, last_mm)."""
                first = last = None
                cnt = 0
                while input_ops and cnt < n:
                    mm = input_ops.pop(0)()
                    cnt += 1
                    if mm is not None:
                        if first is None:
                            first = mm
                        last = mm
                return first, last

            # ---- head: produce the first HEAD_CH chunks ----------------
            n_head = min(HEAD_CH, n_chunks)
            head_ops = n_head * (1 + MT * (KT + 1))
            f_h, l_h = pop_input(head_ops)
            seg_edge(f_h, l_h)

            # ---- persistent h state ------------------------------------
            # per-group ping-pong tiles; bf16 copy feeds the PE, fp32 is
            # the exact accumulator (gpsimd) and the final output.
            CA = GROUPS[0][1] - GROUPS[0][0]   # 4 chunks
            CB = GROUPS[1][1] - GROUPS[1][0]   # 2 chunks
            h_bA = [h_pool.tile([128, CA, BQ], BF16_T, name=f"hbA{i}", tag=f"hbA{i}") for i in range(2)]
            h_bB = [h_pool.tile([128, CB, BQ], BF16_T, name=f"hbB{i}", tag=f"hbB{i}") for i in range(2)]
            h_fA = [h_pool.tile([128, CA, BQ], F32, name=f"hfA{i}", tag=f"hfA{i}") for i in range(2)]
            h_fB = [h_pool.tile([128, CB, BQ], F32, name=f"hfB{i}", tag=f"hfB{i}") for i in range(2)]
            nc.vector.memset(h_bA[0][:], 0.0)
            nc.vector.memset(h_bB[0][:], 0.0)
            nc.gpsimd.memset(h_fA[0][:], 0.0)
            nc.gpsimd.memset(h_fB[0][:], 0.0)

            gi_blks = {}

            # gi_blk column layout per step: for each group [rz(2*nch) | n(nch)]
            # so the identity-matmul's moving operand and sigma's gi are
            # contiguous 2D slices.  slot(m): m=(c,g) -> group base + 2*cl+g
            # (rz) or base + 2*nch + cl (n).
            def gi_slot(m):
                c, g = m // 3, m % 3
                for (c0, c1) in GROUPS:
                    if c0 <= c < c1:
                        base, cl, nch = 3 * c0, c - c0, c1 - c0
                        return base + (2 * cl + g if g < 2 else 2 * nch + cl)
                raise AssertionError

            def load_block(b):
                gi = giblk_pool.tile([128, BLK, MT * BQ], BF16_T)
                gi_blks[b] = gi
                for m in range(MT):
                    sl = gi_slot(m)
                    nc.sync.dma_start(
                        gi[:, :, sl * BQ:(sl + 1) * BQ],
                        giD[m, :, b * BLK * BQ:(b + 1) * BLK * BQ].rearrange(
                            "p (t b) -> p t b", b=BQ
                        ),
                    )

            load_block(0)
            if n_blocks > 1:
                load_block(1)

            def h_src(group_idx, k, which):
                """moving-operand slice of h for contraction k-tile k."""
                if k < CA:
                    return (h_bA if which == "b" else h_fA)[group_idx][:, k, :]
                return (h_bB if which == "b" else h_fB)[group_idx][:, k - CA, :]

            # ---- recurrence --------------------------------------------
            # Per step, in PE order: [A-burst][input quanta][B-burst].
            # Group A's gate chain (DVE + ACT) starts when A's psums close
            # (mid-burst) and overlaps the input + B work; group B's chain
            # runs on GpSimd + ACT and overlaps the next step's A-burst.
            for t in range(steps):
                cur, nxt = t % 2, (t + 1) % 2
                blk, tl = t // BLK, t % BLK

                if t % BLK == 0 and blk + 2 < n_blocks:
                    load_block(blk + 2)
                if t % BLK == 0 and blk - 2 in gi_blks:
                    del gi_blks[blk - 2]
                gi_blk = gi_blks[blk]

                for gidx, (c0, c1) in enumerate(GROUPS):
                    nch_g = c1 - c0
                    ve = nc.vector if gidx == 0 else nc.gpsimd
                    # one flat psum tile per group: cols [0:2*nch*BQ] hold rz
                    # (ci,g)-interleaved, cols [2*nch*BQ:] hold n -- sigma and
                    # the identity-matmul see contiguous 2D blocks.
                    pool_g = psA_pool if gidx == 0 else psB_pool
                    nrz = 2 * nch_g * BQ
                    ps = pool_g.tile([128, 3 * nch_g * BQ], F32, name=f"ps{gidx}")

                    base = 3 * c0 * BQ
                    gi_rz = gi_blk[:, tl, base:base + nrz]
                    gi_n = gi_blk[:, tl, base + nrz:base + 3 * nch_g * BQ]
                    bn3 = bn_t[:, c0 * BQ:c1 * BQ]

                    # burst: identity-mm seeds the rz block with gi_rz, then
                    # the nch*3 gates x 6 k recurrent matmuls accumulate gh.
                    use_id = os.environ.get("KNOID", "1") != "1"
                    first = last = None
                    if use_id:
                        first = last = nc.tensor.matmul(
                            ps[:, 0:nrz],
                            ident_t[:],
                            gi_rz,
                            start=True, stop=False,
                            skip_group_check=True,
                        )
                    for ci in range(nch_g):
                        for g in range(3):
                            m = (c0 + ci) * 3 + g
                            sl = (2 * ci + g if g < 2 else 2 * nch_g + ci)
                            for k in range(KT):
                                mm = nc.tensor.matmul(
                                    ps[:, sl * BQ:(sl + 1) * BQ],
                                    whh_t[:, k * W3D + m * 128:k * W3D + (m + 1) * 128],
                                    h_src(cur, k, "b"),
                                    start=(k == 0 and (g == 2 or not use_id)),
                                    stop=(k == KT - 1),
                                    skip_group_check=True,
                                )
                                if use_id and k == 0 and g < 2:
                                    # accumulating writes commute for the
                                    # scheduler, but the identity-mm's
                                    # start=True reset must precede them
                                    tile.add_dep_helper(
                                        mm.ins, first.ins, sync=False,
                                        reason="id-mm seeds psum first")
                                if first is None:
                                    first = mm
                                last = mm
                    seg_edge(first, last)

                    # gate chain (A: DVE, B: GpSimd; sigma/tanh on ACT)
                    h_b_n = (h_bA if gidx == 0 else h_bB)[nxt]
                    h_f_c = (h_fA if gidx == 0 else h_fB)[cur]
                    h_f_n = (h_fA if gidx == 0 else h_fB)[nxt]

                    rz = tmp.tile([128, 2 * nch_g * BQ], F32, tag=f"rz{gidx}")
                    if use_id:
                        sig_in = ps[:, 0:nrz]
                    else:
                        arz = tmp.tile([128, 2 * nch_g * BQ], F32, tag=f"arz{gidx}")
                        nc.vector.tensor_add(arz[:], ps[:, 0:nrz], gi_rz)
                        sig_in = arz[:]
                    sig = nc.scalar.activation(rz[:], sig_in,
                                               AF.Sigmoid, scale=inv_s)
                    act_edge(sig)
                    rz4 = rz[:].rearrange("p (c g b) -> p c g b", g=2, b=BQ)
                    r_s, z_s = rz4[:, :, 0, :], rz4[:, :, 1, :]
                    # gpsimd cannot read PSUM -- hnb always on DVE (it runs
                    # during sigma, when DVE is otherwise idle)
                    hnb = tmp.tile([128, nch_g, BQ], F32, tag=f"hn{gidx}")
                    nc.vector.tensor_add(
                        hnb[:], ps[:, nrz:].rearrange("p (c b) -> p c b", b=BQ),
                        bn3.rearrange("p (c b) -> p c b", b=BQ))
                    zh = tmp.tile([128, nch_g, BQ], F32, tag=f"zh{gidx}")
                    nc.gpsimd.tensor_mul(zh[:], z_s, h_f_c[:])
                    u = tmp.tile([128, nch_g, BQ], F32, tag=f"u{gidx}")
                    ve.tensor_mul(u[:], r_s, hnb[:])
                    v = tmp.tile([128, nch_g, BQ], F32, tag=f"v{gidx}")
                    ve.tensor_add(v[:], u[:], gi_n.rearrange("p (c b) -> p c b", b=BQ))
                    n_t = tmp.tile([128, nch_g, BQ], F32, tag=f"n{gidx}")
                    tah = nc.scalar.activation(n_t[:], v[:], AF.Tanh, scale=inv_s)
                    act_edge(tah)
                    q = tmp.tile([128, nch_g, BQ], F32, tag=f"q{gidx}")
                    ve.tensor_scalar(q[:], z_s, -1.0, 1.0,
                                     mybir.AluOpType.mult, mybir.AluOpType.add)
                    w_t = tmp.tile([128, nch_g, BQ], F32, tag=f"w{gidx}")
                    ve.tensor_mul(w_t[:], q[:], n_t[:])
                    ve.tensor_add(h_b_n[:], w_t[:], zh[:])
                    # fp32 state update off the critical chain
                    (nc.gpsimd if gidx == 0 else nc.vector).tensor_add(
                        h_f_n[:], w_t[:], zh[:])

                    if gidx == 0:
                        # input quanta ride between the A and B bursts
                        f_in, l_in = pop_input(IN_Q)
                        seg_edge(f_in, l_in)

            # drain any remaining input work (only for tiny debug step counts)
            while input_ops:
                pop_input(len(input_ops))

            fin = steps % 2
            nc.sync.dma_start(
                t_out.ap().rearrange("p (c b) -> p c b", b=BQ)[:, 0:CA, :],
                h_fA[fin][:])
            nc.sync.dma_start(
                t_out.ap().rearrange("p (c b) -> p c b", b=BQ)[:, CA:KT, :],
                h_fB[fin][:])

    nc.compile()
    return nc


def _pack_encoder(emb, Wih, Whh, bih, bhh):
    """Host-side prep of one encoder's parameters into device layouts."""
    emb_bf = np.ascontiguousarray(emb.astype(BF16))
    Wp = (Wih[_PERM] * SCALE)  # [2304, 768], pre-scaled
    wihT = np.ascontiguousarray(
        Wp.reshape(3 * D, KT, 128).transpose(2, 1, 0).reshape(128, KT * 3 * D).astype(BF16)
    )
    Wp = Whh[_PERM] * SCALE
    whhT_f = Wp.reshape(3 * D, KT, 128).transpose(2, 1, 0).reshape(128, KT * 3 * D)
    if MODE == "fp8":
        whhT = np.ascontiguousarray(whhT_f.astype(FP8E4))
    else:
        whhT = np.ascontiguousarray(whhT_f.astype(BF16))
    bias_vec = (bih + np.concatenate([bhh[:D], bhh[D:2 * D], np.zeros(D, np.float32)]))[_PERM] * SCALE
    bias_i = np.ascontiguousarray(bias_vec.reshape(MT, 128).T.astype(np.float32))
    bhh_n = bhh[2 * D:] * SCALE
    bhhn = np.ascontiguousarray(
        np.repeat(bhh_n.reshape(KT, 128).T[:, :, None], BQ, axis=2).reshape(128, KT * BQ).astype(np.float32)
    )
    return emb_bf, wihT, whhT, bias_i, bhhn


_CACHE = {}


def run_device(inputs, steps=S, trace=False):
    """Run the 8-core device program; returns (h_ctx [64,768], h_tgt [64,768], perf)."""
    key = (steps, MODE)
    if key not in _CACHE:
        _CACHE[key] = _build_program(steps)
    nc = _CACHE[key]

    ctx_tok = np.asarray(inputs["ctx"]).astype(np.int16)      # [64, 256]
    tgt_tok = np.asarray(inputs["tgt_seq"]).astype(np.int16)  # [64, 256]

    enc_ctx = _pack_encoder(
        np.asarray(inputs["emb"], np.float32), np.asarray(inputs["Wih"], np.float32),
        np.asarray(inputs["Whh"], np.float32), np.asarray(inputs["bih"], np.float32),
        np.asarray(inputs["bhh"], np.float32),
    )
    enc_tgt = _pack_encoder(
        np.asarray(inputs["t_emb"], np.float32), np.asarray(inputs["t_Wih"], np.float32),
        np.asarray(inputs["t_Whh"], np.float32), np.asarray(inputs["t_bih"], np.float32),
        np.asarray(inputs["t_bhh"], np.float32),
    )

    in_maps = []
    for core in range(N_CORES):
        e, q = core // 4, core % 4
        emb_bf, wihT, whhT, bias_i, bhhn = enc_ctx if e == 0 else enc_tgt
        toks = (ctx_tok if e == 0 else tgt_tok)[q * BQ:(q + 1) * BQ, :]  # [16, 256]
        # gather position i = t*16+b reads idx[i%16, i//16] = toks[b, t]; the
        # [16, NT/16] block must be replicated into each gpsimd core's stripe.
        idx = np.tile(toks, (8, 1))
        in_maps.append({
            "ident": np.eye(128, dtype=np.float32).astype(BF16),
            "idx": idx,
            "emb": emb_bf,
            "wihT": wihT,
            "whhT": whhT,
            "bias_i": bias_i,
            "bhhn": bhhn,
        })

    res = run_bass_kernel_spmd(nc, in_maps, core_ids=list(range(N_CORES)), trace=trace)

    def unpack_h(outs):
        # out [128, KT*BQ]: out[p, k*BQ + b] = h[b, k*128 + p]
        h = np.zeros((4 * BQ, D), np.float32)
        for q in range(4):
            o = np.asarray(outs[q]["h_out"], np.float32).reshape(128, KT, BQ)
            h[q * BQ:(q + 1) * BQ, :] = o.transpose(2, 1, 0).reshape(BQ, D)
        return h

    h_ctx = unpack_h(res.results[0:4])
    h_tgt = unpack_h(res.results[4:8])
    return h_ctx, h_tgt, res


def _head(h_ctx, h_tgt, inputs):
    """Final tiny math on host, float64 for exactness."""
    Wfc = np.asarray(inputs["Wfc"], np.float64)
    bfc = np.asarray(inputs["bfc"], np.float64)
    tWfc = np.asarray(inputs["t_Wfc"], np.float64)
    tbfc = np.asarray(inputs["t_bfc"], np.float64)
    We = np.asarray(inputs["We"], np.float64)
    u0 = np.asarray(inputs["u_sn"], np.float64)

    ctx_latent = h_ctx.astype(np.float64) @ Wfc.T + bfc          # [64, 8]
    target_latent = h_tgt.astype(np.float64) @ tWfc.T + tbfc     # [64, 8]

    u = u0 / (np.linalg.norm(u0) + 1e-12)
    for _ in range(PI):
        v = We.T @ u
        v = v / (np.linalg.norm(v) + 1e-12)
        u = We @ v
        u = u / (np.linalg.norm(u) + 1e-12)
    sigma = u @ (We @ v)
    Wsn = We / sigma

    pred_latent = -(STEPS_DESC * DT_STEP) * (ctx_latent @ Wsn.T)  # [64, 8]
    return (
        pred_latent.astype(np.float32)[:, None, :],
        target_latent.astype(np.float32)[:, None, :],
    )


def kernel(**inputs):
    h_ctx, h_tgt, _ = run_device(inputs, steps=S, trace=False)
    return _head(h_ctx, h_tgt, inputs)


# revision 33
# speedup vs baseline: 1.2967x; 1.2967x over previous
"""Trainium2 Bass kernel for nn_CliffordJEPAModel.

Model = two GRU encoders (ctx / tgt) + tiny closed-form head.

Sharding: 8 cores = 2 encoders x 4 batch-quarters (B_local=16), no
cross-core communication.  The head (fc -> spectral norm -> closed-form
energy descent) runs on host in float64.

Device program (identical on all 8 cores, only data differs):
  * The recurrent matmul gh^T = Whh' @ h^T is LDWEIGHTS-bound on the PE
    array (108 weight tiles of 128x128 per step, ~27ns each in bf16).
    Whh is stored in fp8e4 (e4m3) to double the weight-ingest rate.
    Numerics: Whh is pre-scaled by S=8192 (power of two) on host so its
    values occupy e4m3's normal range; gi / biases are pre-scaled by S
    too, and the sigmoid/tanh activations descale via their input scale
    (= 1/S).  Everything after the activations is unscaled.  Validated
    in numpy: latent rel-err ~7e-3 (budget 2e-2).
  * Gate math is split into two UNEVEN psum groups (A = d-chunks 0-3,
    B = chunks 4-5).  Group A's psum completes at ~2/3 of the burst, so
    its DVE/ACT gate chain overlaps the rest of the burst instead of
    serializing after it (the v1 kernel lost ~2.8us/step to that).
  * The input matmul gi^T = Wih' @ X^T (stream-bound, N=512) is chopped
    into small quanta interleaved at the top of each recurrence step, so
    the phase-1 work rides in the h-latency slack of the step pipeline
    instead of running as a serial prefix.
"""

import os
import sys

for _p in ("/opt/trn_rl_repo/concourse", "/opt/trn_rl_repo"):
    if _p not in sys.path:
        sys.path.insert(0, _p)

import numpy as np
import ml_dtypes

import concourse.bacc as bacc
import concourse.mybir as mybir
import concourse.tile as tile
from concourse.bass_utils import run_bass_kernel_spmd

BF16 = ml_dtypes.bfloat16
FP8E4 = ml_dtypes.float8_e4m3  # concourse float8e4 <-> ml_dtypes.float8_e4m3 (max 240)

V, D, NB = 32000, 768, 8
B, S = 64, 256
DT_STEP, STEPS_DESC, PI = 0.1, 5, 3

N_CORES = 8
BQ = B // 4          # batch rows per core (16)
KT = D // 128        # 6 k-tiles
MT = 3 * KT          # 18 m-tiles of gate rows
NT = BQ * S          # tokens per core (4096)
CHT = 512            # tokens per gather/input-matmul chunk
NCH = NT // CHT      # 8 chunks
BLK = 16             # recurrence steps per gi prefetch block
HEAD_CH = 2          # chunks produced before the step loop starts
IN_Q = 5             # interleaved input-phase quanta per step

# Whh quantization mode: 'fp8' (e4m3 + S scaling) or 'bf16'.  Measured:
# LDWEIGHTS is row-rate-limited (27ns for 128 rows regardless of dtype),
# so fp8 weights buy nothing -- default to bf16 for accuracy.
MODE = os.environ.get("KMODE", "bf16")
SCALE = 4096.0 if MODE == "fp8" else 1.0  # max |Whh|*S ~ 148 < 240 (e4m3 max)

# equal gate groups: A = chunks 0..2, B = chunks 3..5.  A's chain runs on
# DVE while B's burst occupies the PE; B's chain runs on GpSimd.
GROUPS = ((0, 3), (3, 6))

F32 = mybir.dt.float32
BF16_T = mybir.dt.bfloat16
FP8_T = mybir.dt.float8e4
I16 = mybir.dt.int16
AF = mybir.ActivationFunctionType

WHH_T = FP8_T if MODE == "fp8" else BF16_T

# gate-row permutation: m-tile j = (chunk c=j//3, gate g=j%3) covers rows
# g*768 + c*128 .. +128  ->  interleaved [r_c, z_c, n_c] blocks.
_PERM = np.concatenate(
    [np.arange(g * D + c * 128, g * D + (c + 1) * 128) for c in range(KT) for g in range(3)]
)


def _build_program(steps=S):
    nc = bacc.Bacc("TRN2", target_bir_lowering=False, debug=False, num_devices=N_CORES)

    t_ident = nc.dram_tensor("ident", [128, 128], BF16_T, kind="ExternalInput")
    t_idx = nc.dram_tensor("idx", [128, NT // 16], I16, kind="ExternalInput")
    t_emb = nc.dram_tensor("emb", [V, D], BF16_T, kind="ExternalInput")
    t_wih = nc.dram_tensor("wihT", [128, KT * 3 * D], BF16_T, kind="ExternalInput")
    t_whh = nc.dram_tensor("whhT", [128, KT * 3 * D], WHH_T, kind="ExternalInput")
    t_bi = nc.dram_tensor("bias_i", [128, MT], F32, kind="ExternalInput")
    t_bn = nc.dram_tensor("bhhn", [128, KT * BQ], F32, kind="ExternalInput")
    t_out = nc.dram_tensor("h_out", [128, KT * BQ], F32, kind="ExternalOutput")

    W3D = 3 * D  # 2304
    n_blocks = steps // BLK
    n_chunks = max(1, (steps * BQ) // CHT)
    inv_s = 1.0 / SCALE

    with tile.TileContext(nc) as tc:
        with (
            tc.tile_pool(name="const", bufs=1) as const_pool,
            tc.tile_pool(name="dram", bufs=1, space="DRAM") as dram_pool,
            tc.tile_pool(name="xt", bufs=2) as xt_pool,
            tc.tile_pool(name="psum_in", bufs=2, space="PSUM") as psum_in,
            tc.tile_pool(name="gis", bufs=3) as gis_pool,
            tc.tile_pool(name="psA", bufs=2, space="PSUM") as psA_pool,
            tc.tile_pool(name="psB", bufs=2, space="PSUM") as psB_pool,
            tc.tile_pool(name="giblk", bufs=2) as giblk_pool,
            tc.tile_pool(name="hstate", bufs=1) as h_pool,
            tc.tile_pool(name="tmp", bufs=3) as tmp,
        ):
            ident_t = const_pool.tile([128, 128], BF16_T)
            nc.sync.dma_start(ident_t[:], t_ident.ap())
            idx_t = const_pool.tile([128, NT // 16], I16)
            wih_t = const_pool.tile([128, KT * W3D], BF16_T)
            whh_t = const_pool.tile([128, KT * W3D], WHH_T)
            bi_t = const_pool.tile([128, MT], F32)
            bn_t = const_pool.tile([128, KT * BQ], F32)
            nc.sync.dma_start(idx_t[:], t_idx.ap())
            nc.sync.dma_start(wih_t[:], t_wih.ap())
            nc.sync.dma_start(whh_t[:], t_whh.ap())
            nc.sync.dma_start(bi_t[:], t_bi.ap())
            nc.sync.dma_start(bn_t[:], t_bn.ap())

            giD = dram_pool.tile([MT, 128, NT], BF16_T)

            # ---- scheduler ordering helpers ----------------------------
            prev_pe_last = None   # last PE instr of previous segment

            def seg_edge(first_mm, last_mm):
                nonlocal prev_pe_last
                if first_mm is None:
                    return
                if prev_pe_last is not None:
                    tile.add_dep_helper(first_mm.ins, prev_pe_last.ins,
                                        sync=False, reason="pe segment order")
                prev_pe_last = last_mm

            act_prev = None

            def act_edge(op):
                nonlocal act_prev
                if act_prev is not None:
                    tile.add_dep_helper(op.ins, act_prev.ins, sync=False,
                                        reason="act order")
                act_prev = op

            # ---- input phase as a flat list of closures -----------------
            # each closure emits a tiny slice of (gather | matmul | act+store)
            # work and returns the matmul instruction if it emitted one.
            xt_tiles = {}
            ps_in_tiles = {}

            def mk_gather(nch):
                def f():
                    xt = xt_pool.tile([128, KT, CHT], BF16_T, name=f"xt{nch % 2}")
                    xt_tiles[nch] = xt
                    nc.gpsimd.dma_gather(
                        xt[:, :, :],
                        t_emb.ap(),
                        idx_t[:, nch * (CHT // 16):(nch + 1) * (CHT // 16)],
                        num_idxs=CHT,
                        num_idxs_reg=CHT,
                        elem_size=D,
                        transpose=True,
                    )
                    return None
                return f

            def mk_mm(nch, m, k):
                def f():
                    if k == 0:
                        ps_in_tiles[nch] = ps_in_tiles.get(nch, {})
                        ps_in_tiles[nch][m] = psum_in.tile([128, CHT], F32, name="ps_in")
                    mm = nc.tensor.matmul(
                        ps_in_tiles[nch][m][:],
                        wih_t[:, k * W3D + m * 128:k * W3D + (m + 1) * 128],
                        xt_tiles[nch][:, k, :],
                        start=(k == 0),
                        stop=(k == KT - 1),
                    )
                    return mm
                return f

            def mk_store(nch, m):
                def f():
                    gs = gis_pool.tile([128, CHT], BF16_T)
                    act = nc.scalar.activation(gs[:], ps_in_tiles[nch][m][:], AF.Identity,
                                               bias=bi_t[:, m:m + 1], scale=1.0)
                    act_edge(act)
                    nc.sync.dma_start(giD[m, :, nch * CHT:(nch + 1) * CHT], gs[:])
                    del ps_in_tiles[nch][m]
                    return None
                return f

            input_ops = []
            for nch in range(n_chunks):
                input_ops.append(mk_gather(nch))
                for m in range(MT):
                    for k in range(KT):
                        input_ops.append(mk_mm(nch, m, k))
                    input_ops.append(mk_store(nch, m))

            def pop_input(n):
                """Emit up to n input closures; return (first_mm, last_mm)."""
                first = last = None
                cnt = 0
                while input_ops and cnt < n:
                    mm = input_ops.pop(0)()
                    cnt += 1
                    if mm is not None:
                        if first is None:
                            first = mm
                        last = mm
                return first, last

            # ---- head: produce the first HEAD_CH chunks ----------------
            n_head = min(HEAD_CH, n_chunks)
            head_ops = n_head * (1 + MT * (KT + 1))
            f_h, l_h = pop_input(head_ops)
            seg_edge(f_h, l_h)

            # ---- persistent h state ------------------------------------
            # per-group ping-pong tiles; bf16 copy feeds the PE, fp32 is
            # the exact accumulator (gpsimd) and the final output.
            CA = GROUPS[0][1] - GROUPS[0][0]   # 4 chunks
            CB = GROUPS[1][1] - GROUPS[1][0]   # 2 chunks
            h_bA = [h_pool.tile([128, CA, BQ], BF16_T, name=f"hbA{i}", tag=f"hbA{i}") for i in range(2)]
            h_bB = [h_pool.tile([128, CB, BQ], BF16_T, name=f"hbB{i}", tag=f"hbB{i}") for i in range(2)]
            h_fA = [h_pool.tile([128, CA, BQ], F32, name=f"hfA{i}", tag=f"hfA{i}") for i in range(2)]
            h_fB = [h_pool.tile([128, CB, BQ], F32, name=f"hfB{i}", tag=f"hfB{i}") for i in range(2)]
            nc.vector.memset(h_bA[0][:], 0.0)
            nc.vector.memset(h_bB[0][:], 0.0)
            nc.gpsimd.memset(h_fA[0][:], 0.0)
            nc.gpsimd.memset(h_fB[0][:], 0.0)

            gi_blks = {}

            # gi_blk column layout per step: for each group [rz(2*nch) | n(nch)]
            # so the identity-matmul's moving operand and sigma's gi are
            # contiguous 2D slices.  slot(m): m=(c,g) -> group base + 2*cl+g
            # (rz) or base + 2*nch + cl (n).
            def gi_slot(m):
                c, g = m // 3, m % 3
                for (c0, c1) in GROUPS:
                    if c0 <= c < c1:
                        base, cl, nch = 3 * c0, c - c0, c1 - c0
                        return base + (2 * cl + g if g < 2 else 2 * nch + cl)
                raise AssertionError

            def load_block(b):
                gi = giblk_pool.tile([128, BLK, MT * BQ], BF16_T)
                gi_blks[b] = gi
                for m in range(MT):
                    sl = gi_slot(m)
                    nc.sync.dma_start(
                        gi[:, :, sl * BQ:(sl + 1) * BQ],
                        giD[m, :, b * BLK * BQ:(b + 1) * BLK * BQ].rearrange(
                            "p (t b) -> p t b", b=BQ
                        ),
                    )

            load_block(0)
            if n_blocks > 1:
                load_block(1)

            def h_src(group_idx, k, which):
                """moving-operand slice of h for contraction k-tile k."""
                if k < CA:
                    return (h_bA if which == "b" else h_fA)[group_idx][:, k, :]
                return (h_bB if which == "b" else h_fB)[group_idx][:, k - CA, :]

            # ---- recurrence --------------------------------------------
            # Per step, PE order: [input quanta][A-burst][B-burst], bursts
            # k-outermost so next-step matmuls unblock per h chunk-group.
            # Both gate chains run on DVE+ACT with baseline's proven
            # interleaved ordering; gpsimd takes zh and the fp32 state.
            for t in range(steps):
                cur, nxt = t % 2, (t + 1) % 2
                blk, tl = t // BLK, t % BLK

                if t % BLK == 0 and blk + 2 < n_blocks:
                    load_block(blk + 2)
                if t % BLK == 0 and blk - 2 in gi_blks:
                    del gi_blks[blk - 2]
                gi_blk = gi_blks[blk]

                f_in, l_in = pop_input(IN_Q)
                seg_edge(f_in, l_in)

                ops = {}
                for gidx, (c0, c1) in enumerate(GROUPS):
                    nch_g = c1 - c0
                    pool_g = psA_pool if gidx == 0 else psB_pool
                    nrz = 2 * nch_g * BQ
                    ps = pool_g.tile([128, 3 * nch_g * BQ], F32, name=f"ps{gidx}")
                    ops[f"ps{gidx}"] = ps
                    ops[f"girz{gidx}"] = gi_blk[:, tl, 3 * c0 * BQ:3 * c0 * BQ + nrz]
                    ops[f"gin{gidx}"] = gi_blk[
                        :, tl, 3 * c0 * BQ + nrz:3 * c1 * BQ]
                    ops[f"bn{gidx}"] = bn_t[:, c0 * BQ:c1 * BQ]

                    # burst: the identity-mm is the psum group's single
                    # start (start=True resets has_written for the whole
                    # bank, so exactly one start per tile is legal); it
                    # seeds the rz block with gi_rz.  The recurrent matmuls
                    # run k-outermost (start=False: first write to each
                    # element overwrites, later ones accumulate) so
                    # next-step matmuls unblock as each h chunk lands.
                    first = nc.tensor.matmul(
                        ps[:, 0:nrz],
                        ident_t[:],
                        ops[f"girz{gidx}"],
                        start=True, stop=False,
                        skip_group_check=True,
                    )
                    last = first
                    n_mm = 3 * nch_g * KT
                    cnt = 0
                    for k in range(KT):
                        for ci in range(nch_g):
                            for g in range(3):
                                m = (c0 + ci) * 3 + g
                                sl = (2 * ci + g if g < 2 else 2 * nch_g + ci)
                                cnt += 1
                                mm = nc.tensor.matmul(
                                    ps[:, sl * BQ:(sl + 1) * BQ],
                                    whh_t[:, k * W3D + m * 128:k * W3D + (m + 1) * 128],
                                    h_src(cur, k, "b"),
                                    start=False,
                                    stop=(cnt == n_mm),
                                    skip_group_check=True,
                                )
                                tile.add_dep_helper(
                                    mm.ins, first.ins, sync=False,
                                    reason="group start first")
                                last = mm
                    seg_edge(first, last)

                for gidx, (c0, c1) in enumerate(GROUPS):
                    nch_g = c1 - c0
                    nrz = 2 * nch_g * BQ
                    ps = ops[f"ps{gidx}"]
                    h_b_n = (h_bA if gidx == 0 else h_bB)[nxt]
                    h_f_c = (h_fA if gidx == 0 else h_fB)[cur]
                    h_f_n = (h_fA if gidx == 0 else h_fB)[nxt]

                    rz = tmp.tile([128, nrz], F32, tag=f"rz{gidx}")
                    ops[f"sig{gidx}"] = nc.scalar.activation(
                        rz[:], ps[:, 0:nrz], AF.Sigmoid, scale=inv_s)
                    rz4 = rz[:].rearrange("p (c g b) -> p c g b", g=2, b=BQ)
                    r_s, z_s = rz4[:, :, 0, :], rz4[:, :, 1, :]
                    hnb = tmp.tile([128, nch_g, BQ], F32, tag=f"hn{gidx}")
                    ops[f"hn{gidx}"] = nc.vector.tensor_add(
                        hnb[:], ps[:, nrz:].rearrange("p (c b) -> p c b", b=BQ),
                        ops[f"bn{gidx}"].rearrange("p (c b) -> p c b", b=BQ))
                    zh = tmp.tile([128, nch_g, BQ], F32, tag=f"zh{gidx}")
                    nc.gpsimd.tensor_mul(zh[:], z_s, h_f_c[:])
                    u = tmp.tile([128, nch_g, BQ], F32, tag=f"u{gidx}")
                    ops[f"u{gidx}"] = nc.vector.tensor_mul(u[:], r_s, hnb[:])
                    v = tmp.tile([128, nch_g, BQ], F32, tag=f"v{gidx}")
                    ops[f"v{gidx}"] = nc.vector.tensor_add(
                        v[:], u[:],
                        ops[f"gin{gidx}"].rearrange("p (c b) -> p c b", b=BQ))
                    n_t = tmp.tile([128, nch_g, BQ], F32, tag=f"n{gidx}")
                    ops[f"tanh{gidx}"] = nc.scalar.activation(
                        n_t[:], v[:], AF.Tanh, scale=inv_s)
                    q = tmp.tile([128, nch_g, BQ], F32, tag=f"q{gidx}")
                    ops[f"q{gidx}"] = nc.vector.tensor_scalar(
                        q[:], z_s, -1.0, 1.0,
                        mybir.AluOpType.mult, mybir.AluOpType.add)
                    w_t = tmp.tile([128, nch_g, BQ], F32, tag=f"w{gidx}")
                    ops[f"w{gidx}"] = nc.vector.tensor_mul(w_t[:], q[:], n_t[:])
                    ops[f"hb{gidx}"] = nc.vector.tensor_add(
                        h_b_n[:], w_t[:], zh[:])
                    nc.gpsimd.tensor_add(h_f_n[:], w_t[:], zh[:])

                # enforce the engine orders that keep both chains tight
                dve_order = ["hn0", "u0", "v0", "q0",
                             "hn1", "u1", "v1", "w0", "hb0", "q1",
                             "w1", "hb1"]
                for a, b in zip(dve_order, dve_order[1:]):
                    tile.add_dep_helper(ops[b].ins, ops[a].ins, sync=False,
                                        reason="dve chain order")
                act_edge(ops["sig0"])
                act_edge(ops["sig1"])
                act_edge(ops["tanh0"])
                act_edge(ops["tanh1"])

            # drain any remaining input work (only for tiny debug step counts)
            while input_ops:
                pop_input(len(input_ops))

            fin = steps % 2
            nc.sync.dma_start(
                t_out.ap().rearrange("p (c b) -> p c b", b=BQ)[:, 0:CA, :],
                h_fA[fin][:])
            nc.sync.dma_start(
                t_out.ap().rearrange("p (c b) -> p c b", b=BQ)[:, CA:KT, :],
                h_fB[fin][:])

    nc.compile()
    return nc


def _pack_encoder(emb, Wih, Whh, bih, bhh):
    """Host-side prep of one encoder's parameters into device layouts."""
    emb_bf = np.ascontiguousarray(emb.astype(BF16))
    Wp = (Wih[_PERM] * SCALE)  # [2304, 768], pre-scaled
    wihT = np.ascontiguousarray(
        Wp.reshape(3 * D, KT, 128).transpose(2, 1, 0).reshape(128, KT * 3 * D).astype(BF16)
    )
    Wp = Whh[_PERM] * SCALE
    whhT_f = Wp.reshape(3 * D, KT, 128).transpose(2, 1, 0).reshape(128, KT * 3 * D)
    if MODE == "fp8":
        whhT = np.ascontiguousarray(whhT_f.astype(FP8E4))
    else:
        whhT = np.ascontiguousarray(whhT_f.astype(BF16))
    bias_vec = (bih + np.concatenate([bhh[:D], bhh[D:2 * D], np.zeros(D, np.float32)]))[_PERM] * SCALE
    bias_i = np.ascontiguousarray(bias_vec.reshape(MT, 128).T.astype(np.float32))
    bhh_n = bhh[2 * D:] * SCALE
    bhhn = np.ascontiguousarray(
        np.repeat(bhh_n.reshape(KT, 128).T[:, :, None], BQ, axis=2).reshape(128, KT * BQ).astype(np.float32)
    )
    return emb_bf, wihT, whhT, bias_i, bhhn


_CACHE = {}


def run_device(inputs, steps=S, trace=False):
    """Run the 8-core device program; returns (h_ctx [64,768], h_tgt [64,768], perf)."""
    key = (steps, MODE)
    if key not in _CACHE:
        _CACHE[key] = _build_program(steps)
    nc = _CACHE[key]

    ctx_tok = np.asarray(inputs["ctx"]).astype(np.int16)      # [64, 256]
    tgt_tok = np.asarray(inputs["tgt_seq"]).astype(np.int16)  # [64, 256]

    enc_ctx = _pack_encoder(
        np.asarray(inputs["emb"], np.float32), np.asarray(inputs["Wih"], np.float32),
        np.asarray(inputs["Whh"], np.float32), np.asarray(inputs["bih"], np.float32),
        np.asarray(inputs["bhh"], np.float32),
    )
    enc_tgt = _pack_encoder(
        np.asarray(inputs["t_emb"], np.float32), np.asarray(inputs["t_Wih"], np.float32),
        np.asarray(inputs["t_Whh"], np.float32), np.asarray(inputs["t_bih"], np.float32),
        np.asarray(inputs["t_bhh"], np.float32),
    )

    in_maps = []
    for core in range(N_CORES):
        e, q = core // 4, core % 4
        emb_bf, wihT, whhT, bias_i, bhhn = enc_ctx if e == 0 else enc_tgt
        toks = (ctx_tok if e == 0 else tgt_tok)[q * BQ:(q + 1) * BQ, :]  # [16, 256]
        # gather position i = t*16+b reads idx[i%16, i//16] = toks[b, t]; the
        # [16, NT/16] block must be replicated into each gpsimd core's stripe.
        idx = np.tile(toks, (8, 1))
        in_maps.append({
            "ident": np.eye(128, dtype=np.float32).astype(BF16),
            "idx": idx,
            "emb": emb_bf,
            "wihT": wihT,
            "whhT": whhT,
            "bias_i": bias_i,
            "bhhn": bhhn,
        })

    res = run_bass_kernel_spmd(nc, in_maps, core_ids=list(range(N_CORES)), trace=trace)

    def unpack_h(outs):
        # out [128, KT*BQ]: out[p, k*BQ + b] = h[b, k*128 + p]
        h = np.zeros((4 * BQ, D), np.float32)
        for q in range(4):
            o = np.asarray(outs[q]["h_out"], np.float32).reshape(128, KT, BQ)
            h[q * BQ:(q + 1) * BQ, :] = o.transpose(2, 1, 0).reshape(BQ, D)
        return h

    h_ctx = unpack_h(res.results[0:4])
    h_tgt = unpack_h(res.results[4:8])
    return h_ctx, h_tgt, res


def _head(h_ctx, h_tgt, inputs):
    """Final tiny math on host, float64 for exactness."""
    Wfc = np.asarray(inputs["Wfc"], np.float64)
    bfc = np.asarray(inputs["bfc"], np.float64)
    tWfc = np.asarray(inputs["t_Wfc"], np.float64)
    tbfc = np.asarray(inputs["t_bfc"], np.float64)
    We = np.asarray(inputs["We"], np.float64)
    u0 = np.asarray(inputs["u_sn"], np.float64)

    ctx_latent = h_ctx.astype(np.float64) @ Wfc.T + bfc          # [64, 8]
    target_latent = h_tgt.astype(np.float64) @ tWfc.T + tbfc     # [64, 8]

    u = u0 / (np.linalg.norm(u0) + 1e-12)
    for _ in range(PI):
        v = We.T @ u
        v = v / (np.linalg.norm(v) + 1e-12)
        u = We @ v
        u = u / (np.linalg.norm(u) + 1e-12)
    sigma = u @ (We @ v)
    Wsn = We / sigma

    pred_latent = -(STEPS_DESC * DT_STEP) * (ctx_latent @ Wsn.T)  # [64, 8]
    return (
        pred_latent.astype(np.float32)[:, None, :],
        target_latent.astype(np.float32)[:, None, :],
    )


def kernel(**inputs):
    h_ctx, h_tgt, _ = run_device(inputs, steps=S, trace=False)
    return _head(h_ctx, h_tgt, inputs)


# revision 34
# speedup vs baseline: 1.3467x; 1.0385x over previous
"""Trainium2 Bass kernel for nn_CliffordJEPAModel.

Model = two GRU encoders (ctx / tgt) + tiny closed-form head.

Sharding: 8 cores = 2 encoders x 4 batch-quarters (B_local=16), no
cross-core communication.  The head (fc -> spectral norm -> closed-form
energy descent) runs on host in float64.

Device program (identical on all 8 cores, only data differs):
  * The recurrent matmul gh^T = Whh' @ h^T is LDWEIGHTS-bound on the PE
    array (108 weight tiles of 128x128 per step, ~27ns each in bf16).
    Whh is stored in fp8e4 (e4m3) to double the weight-ingest rate.
    Numerics: Whh is pre-scaled by S=8192 (power of two) on host so its
    values occupy e4m3's normal range; gi / biases are pre-scaled by S
    too, and the sigmoid/tanh activations descale via their input scale
    (= 1/S).  Everything after the activations is unscaled.  Validated
    in numpy: latent rel-err ~7e-3 (budget 2e-2).
  * Gate math is split into two UNEVEN psum groups (A = d-chunks 0-3,
    B = chunks 4-5).  Group A's psum completes at ~2/3 of the burst, so
    its DVE/ACT gate chain overlaps the rest of the burst instead of
    serializing after it (the v1 kernel lost ~2.8us/step to that).
  * The input matmul gi^T = Wih' @ X^T (stream-bound, N=512) is chopped
    into small quanta interleaved at the top of each recurrence step, so
    the phase-1 work rides in the h-latency slack of the step pipeline
    instead of running as a serial prefix.
"""

import os
import sys

for _p in ("/opt/trn_rl_repo/concourse", "/opt/trn_rl_repo"):
    if _p not in sys.path:
        sys.path.insert(0, _p)

import numpy as np
import ml_dtypes

import concourse.bacc as bacc
import concourse.mybir as mybir
import concourse.tile as tile
from concourse.bass_utils import run_bass_kernel_spmd

BF16 = ml_dtypes.bfloat16
FP8E4 = ml_dtypes.float8_e4m3  # concourse float8e4 <-> ml_dtypes.float8_e4m3 (max 240)

V, D, NB = 32000, 768, 8
B, S = 64, 256
DT_STEP, STEPS_DESC, PI = 0.1, 5, 3

N_CORES = 8
BQ = B // 4          # batch rows per core (16)
KT = D // 128        # 6 k-tiles
MT = 3 * KT          # 18 m-tiles of gate rows
NT = BQ * S          # tokens per core (4096)
CHT = 512            # tokens per gather/input-matmul chunk
NCH = NT // CHT      # 8 chunks
BLK = 16             # recurrence steps per gi prefetch block
HEAD_CH = 2          # chunks produced before the step loop starts
IN_Q = 5             # interleaved input-phase quanta per step

# Whh quantization mode: 'fp8' (e4m3 + S scaling) or 'bf16'.  Measured:
# LDWEIGHTS is row-rate-limited (27ns for 128 rows regardless of dtype),
# so fp8 weights buy nothing -- default to bf16 for accuracy.
MODE = os.environ.get("KMODE", "bf16")
SCALE = 4096.0 if MODE == "fp8" else 1.0  # max |Whh|*S ~ 148 < 240 (e4m3 max)

# equal gate groups: A = chunks 0..2, B = chunks 3..5.  A's chain runs on
# DVE while B's burst occupies the PE; B's chain runs on GpSimd.
GROUPS = ((0, 3), (3, 6))

F32 = mybir.dt.float32
BF16_T = mybir.dt.bfloat16
FP8_T = mybir.dt.float8e4
I16 = mybir.dt.int16
AF = mybir.ActivationFunctionType

WHH_T = FP8_T if MODE == "fp8" else BF16_T

# gate-row permutation: m-tile j = (chunk c=j//3, gate g=j%3) covers rows
# g*768 + c*128 .. +128  ->  interleaved [r_c, z_c, n_c] blocks.
_PERM = np.concatenate(
    [np.arange(g * D + c * 128, g * D + (c + 1) * 128) for c in range(KT) for g in range(3)]
)


def _build_program(steps=S):
    nc = bacc.Bacc("TRN2", target_bir_lowering=False, debug=False, num_devices=N_CORES)

    t_ident = nc.dram_tensor("ident", [128, 128], BF16_T, kind="ExternalInput")
    t_idx = nc.dram_tensor("idx", [128, NT // 16], I16, kind="ExternalInput")
    t_emb = nc.dram_tensor("emb", [V, D], BF16_T, kind="ExternalInput")
    t_wih = nc.dram_tensor("wihT", [128, KT * 3 * D], BF16_T, kind="ExternalInput")
    t_whh = nc.dram_tensor("whhT", [128, KT * 3 * D], WHH_T, kind="ExternalInput")
    t_bi = nc.dram_tensor("bias_i", [128, MT], F32, kind="ExternalInput")
    t_bn = nc.dram_tensor("bhhn", [128, KT * BQ], F32, kind="ExternalInput")
    t_out = nc.dram_tensor("h_out", [128, KT * BQ], F32, kind="ExternalOutput")

    W3D = 3 * D  # 2304
    n_blocks = steps // BLK
    n_chunks = max(1, (steps * BQ) // CHT)
    inv_s = 1.0 / SCALE

    with tile.TileContext(nc) as tc:
        with (
            tc.tile_pool(name="const", bufs=1) as const_pool,
            tc.tile_pool(name="dram", bufs=1, space="DRAM") as dram_pool,
            tc.tile_pool(name="xt", bufs=2) as xt_pool,
            tc.tile_pool(name="psum_in", bufs=2, space="PSUM") as psum_in,
            tc.tile_pool(name="gis", bufs=3) as gis_pool,
            tc.tile_pool(name="psA", bufs=2, space="PSUM") as psA_pool,
            tc.tile_pool(name="psB", bufs=2, space="PSUM") as psB_pool,
            tc.tile_pool(name="giblk", bufs=2) as giblk_pool,
            tc.tile_pool(name="hstate", bufs=1) as h_pool,
            tc.tile_pool(name="tmp", bufs=3) as tmp,
        ):
            ident_t = const_pool.tile([128, 128], BF16_T)
            nc.sync.dma_start(ident_t[:], t_ident.ap())
            idx_t = const_pool.tile([128, NT // 16], I16)
            wih_t = const_pool.tile([128, KT * W3D], BF16_T)
            whh_t = const_pool.tile([128, KT * W3D], WHH_T)
            bi_t = const_pool.tile([128, MT], F32)
            bn_t = const_pool.tile([128, KT * BQ], F32)
            nc.sync.dma_start(idx_t[:], t_idx.ap())
            nc.sync.dma_start(wih_t[:], t_wih.ap())
            nc.sync.dma_start(whh_t[:], t_whh.ap())
            nc.sync.dma_start(bi_t[:], t_bi.ap())
            nc.sync.dma_start(bn_t[:], t_bn.ap())

            giD = dram_pool.tile([MT, 128, NT], BF16_T)

            # ---- scheduler ordering helpers ----------------------------
            prev_pe_last = None   # last PE instr of previous segment

            def seg_edge(first_mm, last_mm):
                nonlocal prev_pe_last
                if first_mm is None:
                    return
                if prev_pe_last is not None:
                    tile.add_dep_helper(first_mm.ins, prev_pe_last.ins,
                                        sync=False, reason="pe segment order")
                prev_pe_last = last_mm

            act_prev = None

            def act_edge(op):
                nonlocal act_prev
                if act_prev is not None:
                    tile.add_dep_helper(op.ins, act_prev.ins, sync=False,
                                        reason="act order")
                act_prev = op

            # ---- input phase as a flat list of closures -----------------
            # each closure emits a tiny slice of (gather | matmul | act+store)
            # work and returns the matmul instruction if it emitted one.
            xt_tiles = {}
            ps_in_tiles = {}

            def mk_gather(nch):
                def f():
                    xt = xt_pool.tile([128, KT, CHT], BF16_T, name=f"xt{nch % 2}")
                    xt_tiles[nch] = xt
                    nc.gpsimd.dma_gather(
                        xt[:, :, :],
                        t_emb.ap(),
                        idx_t[:, nch * (CHT // 16):(nch + 1) * (CHT // 16)],
                        num_idxs=CHT,
                        num_idxs_reg=CHT,
                        elem_size=D,
                        transpose=True,
                    )
                    return None
                return f

            def mk_mm(nch, m, k):
                def f():
                    if k == 0:
                        ps_in_tiles[nch] = ps_in_tiles.get(nch, {})
                        ps_in_tiles[nch][m] = psum_in.tile([128, CHT], F32, name="ps_in")
                    mm = nc.tensor.matmul(
                        ps_in_tiles[nch][m][:],
                        wih_t[:, k * W3D + m * 128:k * W3D + (m + 1) * 128],
                        xt_tiles[nch][:, k, :],
                        start=(k == 0),
                        stop=(k == KT - 1),
                    )
                    return mm
                return f

            def mk_store(nch, m):
                def f():
                    gs = gis_pool.tile([128, CHT], BF16_T)
                    act = nc.scalar.activation(gs[:], ps_in_tiles[nch][m][:], AF.Identity,
                                               bias=bi_t[:, m:m + 1], scale=1.0)
                    act_edge(act)
                    nc.sync.dma_start(giD[m, :, nch * CHT:(nch + 1) * CHT], gs[:])
                    del ps_in_tiles[nch][m]
                    return None
                return f

            input_ops = []
            for nch in range(n_chunks):
                input_ops.append(mk_gather(nch))
                for m in range(MT):
                    for k in range(KT):
                        input_ops.append(mk_mm(nch, m, k))
                    input_ops.append(mk_store(nch, m))

            def pop_input(n):
                """Emit up to n input closures; return (first_mm, last_mm)."""
                first = last = None
                cnt = 0
                while input_ops and cnt < n:
                    mm = input_ops.pop(0)()
                    cnt += 1
                    if mm is not None:
                        if first is None:
                            first = mm
                        last = mm
                return first, last

            # ---- head: produce the first HEAD_CH chunks ----------------
            n_head = min(HEAD_CH, n_chunks)
            head_ops = n_head * (1 + MT * (KT + 1))
            f_h, l_h = pop_input(head_ops)
            seg_edge(f_h, l_h)

            # ---- persistent h state ------------------------------------
            # per-group ping-pong tiles; bf16 copy feeds the PE, fp32 is
            # the exact accumulator (gpsimd) and the final output.
            CA = GROUPS[0][1] - GROUPS[0][0]   # 4 chunks
            CB = GROUPS[1][1] - GROUPS[1][0]   # 2 chunks
            h_bA = [h_pool.tile([128, CA, BQ], BF16_T, name=f"hbA{i}", tag=f"hbA{i}") for i in range(2)]
            h_bB = [h_pool.tile([128, CB, BQ], BF16_T, name=f"hbB{i}", tag=f"hbB{i}") for i in range(2)]
            h_fA = [h_pool.tile([128, CA, BQ], F32, name=f"hfA{i}", tag=f"hfA{i}") for i in range(2)]
            h_fB = [h_pool.tile([128, CB, BQ], F32, name=f"hfB{i}", tag=f"hfB{i}") for i in range(2)]
            nc.vector.memset(h_bA[0][:], 0.0)
            nc.vector.memset(h_bB[0][:], 0.0)
            nc.gpsimd.memset(h_fA[0][:], 0.0)
            nc.gpsimd.memset(h_fB[0][:], 0.0)

            gi_blks = {}

            # gi_blk column layout per step: for each group [rz(2*nch) | n(nch)]
            # so the identity-matmul's moving operand and sigma's gi are
            # contiguous 2D slices.  slot(m): m=(c,g) -> group base + 2*cl+g
            # (rz) or base + 2*nch + cl (n).
            def gi_slot(m):
                c, g = m // 3, m % 3
                for (c0, c1) in GROUPS:
                    if c0 <= c < c1:
                        base, cl, nch = 3 * c0, c - c0, c1 - c0
                        return base + (2 * cl + g if g < 2 else 2 * nch + cl)
                raise AssertionError

            def load_block(b):
                gi = giblk_pool.tile([128, BLK, MT * BQ], BF16_T)
                gi_blks[b] = gi
                for m in range(MT):
                    sl = gi_slot(m)
                    nc.sync.dma_start(
                        gi[:, :, sl * BQ:(sl + 1) * BQ],
                        giD[m, :, b * BLK * BQ:(b + 1) * BLK * BQ].rearrange(
                            "p (t b) -> p t b", b=BQ
                        ),
                    )

            load_block(0)
            if n_blocks > 1:
                load_block(1)

            def h_src(group_idx, k, which):
                """moving-operand slice of h for contraction k-tile k."""
                if k < CA:
                    return (h_bA if which == "b" else h_fA)[group_idx][:, k, :]
                return (h_bB if which == "b" else h_fB)[group_idx][:, k - CA, :]

            # ---- recurrence --------------------------------------------
            # Per step, PE order: [input quanta][A-burst][B-burst], bursts
            # k-outermost so next-step matmuls unblock per h chunk-group.
            # Both gate chains run on DVE+ACT with baseline's proven
            # interleaved ordering; gpsimd takes zh and the fp32 state.
            for t in range(steps):
                cur, nxt = t % 2, (t + 1) % 2
                blk, tl = t // BLK, t % BLK

                if t % BLK == 0 and blk + 2 < n_blocks:
                    load_block(blk + 2)
                if t % BLK == 0 and blk - 2 in gi_blks:
                    del gi_blks[blk - 2]
                gi_blk = gi_blks[blk]

                f_in, l_in = pop_input(IN_Q)
                seg_edge(f_in, l_in)

                ops = {}
                for gidx, (c0, c1) in enumerate(GROUPS):
                    nch_g = c1 - c0
                    pool_g = psA_pool if gidx == 0 else psB_pool
                    nrz = 2 * nch_g * BQ
                    ps = pool_g.tile([128, 3 * nch_g * BQ], F32, name=f"ps{gidx}")
                    ops[f"ps{gidx}"] = ps
                    ops[f"girz{gidx}"] = gi_blk[:, tl, 3 * c0 * BQ:3 * c0 * BQ + nrz]
                    ops[f"gin{gidx}"] = gi_blk[
                        :, tl, 3 * c0 * BQ + nrz:3 * c1 * BQ]
                    ops[f"bn{gidx}"] = bn_t[:, c0 * BQ:c1 * BQ]

                    # burst: the identity-mm is the psum group's single
                    # start (start=True resets has_written for the whole
                    # bank, so exactly one start per tile is legal); it
                    # seeds the rz block with gi_rz.  The recurrent matmuls
                    # run k-outermost (start=False: first write to each
                    # element overwrites, later ones accumulate) so
                    # next-step matmuls unblock as each h chunk lands.
                    first = nc.tensor.matmul(
                        ps[:, 0:nrz],
                        ident_t[:],
                        ops[f"girz{gidx}"],
                        start=True, stop=False,
                        skip_group_check=True,
                    )
                    last = first
                    n_mm = 3 * nch_g * KT
                    cnt = 0
                    for k in range(KT):
                        for ci in range(nch_g):
                            for g in range(3):
                                m = (c0 + ci) * 3 + g
                                sl = (2 * ci + g if g < 2 else 2 * nch_g + ci)
                                cnt += 1
                                mm = nc.tensor.matmul(
                                    ps[:, sl * BQ:(sl + 1) * BQ],
                                    whh_t[:, k * W3D + m * 128:k * W3D + (m + 1) * 128],
                                    h_src(cur, k, "b"),
                                    start=False,
                                    stop=(cnt == n_mm),
                                    skip_group_check=True,
                                )
                                tile.add_dep_helper(
                                    mm.ins, first.ins, sync=False,
                                    reason="group start first")
                                last = mm
                    seg_edge(first, last)

                for gidx, (c0, c1) in enumerate(GROUPS):
                    nch_g = c1 - c0
                    nrz = 2 * nch_g * BQ
                    ps = ops[f"ps{gidx}"]
                    h_b_n = (h_bA if gidx == 0 else h_bB)[nxt]
                    h_f_c = (h_fA if gidx == 0 else h_fB)[cur]
                    h_f_n = (h_fA if gidx == 0 else h_fB)[nxt]

                    rz = tmp.tile([128, nrz], F32, tag=f"rz{gidx}")
                    ops[f"sig{gidx}"] = nc.scalar.activation(
                        rz[:], ps[:, 0:nrz], AF.Sigmoid, scale=inv_s)
                    rz4 = rz[:].rearrange("p (c g b) -> p c g b", g=2, b=BQ)
                    r_s, z_s = rz4[:, :, 0, :], rz4[:, :, 1, :]
                    hnb = tmp.tile([128, nch_g, BQ], F32, tag=f"hn{gidx}")
                    ops[f"hn{gidx}"] = nc.vector.tensor_add(
                        hnb[:], ps[:, nrz:].rearrange("p (c b) -> p c b", b=BQ),
                        ops[f"bn{gidx}"].rearrange("p (c b) -> p c b", b=BQ))
                    zh = tmp.tile([128, nch_g, BQ], F32, tag=f"zh{gidx}")
                    nc.gpsimd.tensor_mul(zh[:], z_s, h_f_c[:])
                    u = tmp.tile([128, nch_g, BQ], F32, tag=f"u{gidx}")
                    ops[f"u{gidx}"] = nc.vector.tensor_mul(u[:], r_s, hnb[:])
                    v = tmp.tile([128, nch_g, BQ], F32, tag=f"v{gidx}")
                    ops[f"v{gidx}"] = nc.vector.tensor_add(
                        v[:], u[:],
                        ops[f"gin{gidx}"].rearrange("p (c b) -> p c b", b=BQ))
                    n_t = tmp.tile([128, nch_g, BQ], F32, tag=f"n{gidx}")
                    ops[f"tanh{gidx}"] = nc.scalar.activation(
                        n_t[:], v[:], AF.Tanh, scale=inv_s)
                    q = tmp.tile([128, nch_g, BQ], F32, tag=f"q{gidx}")
                    ops[f"q{gidx}"] = nc.vector.tensor_scalar(
                        q[:], z_s, -1.0, 1.0,
                        mybir.AluOpType.mult, mybir.AluOpType.add)
                    w_t = tmp.tile([128, nch_g, BQ], F32, tag=f"w{gidx}")
                    ops[f"w{gidx}"] = nc.vector.tensor_mul(w_t[:], q[:], n_t[:])
                    ops[f"hb{gidx}"] = nc.vector.tensor_add(
                        h_b_n[:], w_t[:], zh[:])
                    nc.gpsimd.tensor_add(h_f_n[:], w_t[:], zh[:])

                # enforce the engine orders that keep both chains tight:
                # finish A's chain first (it feeds the next burst's first
                # matmuls), then B's.
                dve_order = ["hn0", "u0", "v0", "q0", "w0", "hb0",
                             "hn1", "u1", "v1", "q1", "w1", "hb1"]
                for a, b in zip(dve_order, dve_order[1:]):
                    tile.add_dep_helper(ops[b].ins, ops[a].ins, sync=False,
                                        reason="dve chain order")
                act_edge(ops["sig0"])
                act_edge(ops["tanh0"])
                act_edge(ops["sig1"])
                act_edge(ops["tanh1"])

            # drain any remaining input work (only for tiny debug step counts)
            while input_ops:
                pop_input(len(input_ops))

            fin = steps % 2
            nc.sync.dma_start(
                t_out.ap().rearrange("p (c b) -> p c b", b=BQ)[:, 0:CA, :],
                h_fA[fin][:])
            nc.sync.dma_start(
                t_out.ap().rearrange("p (c b) -> p c b", b=BQ)[:, CA:KT, :],
                h_fB[fin][:])

    nc.compile()
    return nc


def _pack_encoder(emb, Wih, Whh, bih, bhh):
    """Host-side prep of one encoder's parameters into device layouts."""
    emb_bf = np.ascontiguousarray(emb.astype(BF16))
    Wp = (Wih[_PERM] * SCALE)  # [2304, 768], pre-scaled
    wihT = np.ascontiguousarray(
        Wp.reshape(3 * D, KT, 128).transpose(2, 1, 0).reshape(128, KT * 3 * D).astype(BF16)
    )
    Wp = Whh[_PERM] * SCALE
    whhT_f = Wp.reshape(3 * D, KT, 128).transpose(2, 1, 0).reshape(128, KT * 3 * D)
    if MODE == "fp8":
        whhT = np.ascontiguousarray(whhT_f.astype(FP8E4))
    else:
        whhT = np.ascontiguousarray(whhT_f.astype(BF16))
    bias_vec = (bih + np.concatenate([bhh[:D], bhh[D:2 * D], np.zeros(D, np.float32)]))[_PERM] * SCALE
    bias_i = np.ascontiguousarray(bias_vec.reshape(MT, 128).T.astype(np.float32))
    bhh_n = bhh[2 * D:] * SCALE
    bhhn = np.ascontiguousarray(
        np.repeat(bhh_n.reshape(KT, 128).T[:, :, None], BQ, axis=2).reshape(128, KT * BQ).astype(np.float32)
    )
    return emb_bf, wihT, whhT, bias_i, bhhn


_CACHE = {}


def run_device(inputs, steps=S, trace=False):
    """Run the 8-core device program; returns (h_ctx [64,768], h_tgt [64,768], perf)."""
    key = (steps, MODE)
    if key not in _CACHE:
        _CACHE[key] = _build_program(steps)
    nc = _CACHE[key]

    ctx_tok = np.asarray(inputs["ctx"]).astype(np.int16)      # [64, 256]
    tgt_tok = np.asarray(inputs["tgt_seq"]).astype(np.int16)  # [64, 256]

    enc_ctx = _pack_encoder(
        np.asarray(inputs["emb"], np.float32), np.asarray(inputs["Wih"], np.float32),
        np.asarray(inputs["Whh"], np.float32), np.asarray(inputs["bih"], np.float32),
        np.asarray(inputs["bhh"], np.float32),
    )
    enc_tgt = _pack_encoder(
        np.asarray(inputs["t_emb"], np.float32), np.asarray(inputs["t_Wih"], np.float32),
        np.asarray(inputs["t_Whh"], np.float32), np.asarray(inputs["t_bih"], np.float32),
        np.asarray(inputs["t_bhh"], np.float32),
    )

    in_maps = []
    for core in range(N_CORES):
        e, q = core // 4, core % 4
        emb_bf, wihT, whhT, bias_i, bhhn = enc_ctx if e == 0 else enc_tgt
        toks = (ctx_tok if e == 0 else tgt_tok)[q * BQ:(q + 1) * BQ, :]  # [16, 256]
        # gather position i = t*16+b reads idx[i%16, i//16] = toks[b, t]; the
        # [16, NT/16] block must be replicated into each gpsimd core's stripe.
        idx = np.tile(toks, (8, 1))
        in_maps.append({
            "ident": np.eye(128, dtype=np.float32).astype(BF16),
            "idx": idx,
            "emb": emb_bf,
            "wihT": wihT,
            "whhT": whhT,
            "bias_i": bias_i,
            "bhhn": bhhn,
        })

    res = run_bass_kernel_spmd(nc, in_maps, core_ids=list(range(N_CORES)), trace=trace)

    def unpack_h(outs):
        # out [128, KT*BQ]: out[p, k*BQ + b] = h[b, k*128 + p]
        h = np.zeros((4 * BQ, D), np.float32)
        for q in range(4):
            o = np.asarray(outs[q]["h_out"], np.float32).reshape(128, KT, BQ)
            h[q * BQ:(q + 1) * BQ, :] = o.transpose(2, 1, 0).reshape(BQ, D)
        return h

    h_ctx = unpack_h(res.results[0:4])
    h_tgt = unpack_h(res.results[4:8])
    return h_ctx, h_tgt, res


def _head(h_ctx, h_tgt, inputs):
    """Final tiny math on host, float64 for exactness."""
    Wfc = np.asarray(inputs["Wfc"], np.float64)
    bfc = np.asarray(inputs["bfc"], np.float64)
    tWfc = np.asarray(inputs["t_Wfc"], np.float64)
    tbfc = np.asarray(inputs["t_bfc"], np.float64)
    We = np.asarray(inputs["We"], np.float64)
    u0 = np.asarray(inputs["u_sn"], np.float64)

    ctx_latent = h_ctx.astype(np.float64) @ Wfc.T + bfc          # [64, 8]
    target_latent = h_tgt.astype(np.float64) @ tWfc.T + tbfc     # [64, 8]

    u = u0 / (np.linalg.norm(u0) + 1e-12)
    for _ in range(PI):
        v = We.T @ u
        v = v / (np.linalg.norm(v) + 1e-12)
        u = We @ v
        u = u / (np.linalg.norm(u) + 1e-12)
    sigma = u @ (We @ v)
    Wsn = We / sigma

    pred_latent = -(STEPS_DESC * DT_STEP) * (ctx_latent @ Wsn.T)  # [64, 8]
    return (
        pred_latent.astype(np.float32)[:, None, :],
        target_latent.astype(np.float32)[:, None, :],
    )


def kernel(**inputs):
    h_ctx, h_tgt, _ = run_device(inputs, steps=S, trace=False)
    return _head(h_ctx, h_tgt, inputs)


# revision 43
# speedup vs baseline: 1.3834x; 1.0273x over previous
"""Trainium2 Bass kernel for nn_CliffordJEPAModel.

Model = two GRU encoders (ctx / tgt) + tiny closed-form head.

Sharding: 8 cores = 2 encoders x 4 batch-quarters (B_local=16), no
cross-core communication.  The head (fc -> spectral norm -> closed-form
energy descent) runs on host in float64.

Device program (identical on all 8 cores, only data differs):
  * The recurrent matmul gh^T = Whh' @ h^T is LDWEIGHTS-bound on the PE
    array (108 weight tiles of 128x128 per step, ~27ns each in bf16).
    Whh is stored in fp8e4 (e4m3) to double the weight-ingest rate.
    Numerics: Whh is pre-scaled by S=8192 (power of two) on host so its
    values occupy e4m3's normal range; gi / biases are pre-scaled by S
    too, and the sigmoid/tanh activations descale via their input scale
    (= 1/S).  Everything after the activations is unscaled.  Validated
    in numpy: latent rel-err ~7e-3 (budget 2e-2).
  * Gate math is split into two UNEVEN psum groups (A = d-chunks 0-3,
    B = chunks 4-5).  Group A's psum completes at ~2/3 of the burst, so
    its DVE/ACT gate chain overlaps the rest of the burst instead of
    serializing after it (the v1 kernel lost ~2.8us/step to that).
  * The input matmul gi^T = Wih' @ X^T (stream-bound, N=512) is chopped
    into small quanta interleaved at the top of each recurrence step, so
    the phase-1 work rides in the h-latency slack of the step pipeline
    instead of running as a serial prefix.
"""

import os
import sys

for _p in ("/opt/trn_rl_repo/concourse", "/opt/trn_rl_repo"):
    if _p not in sys.path:
        sys.path.insert(0, _p)

import numpy as np
import ml_dtypes

import concourse.bacc as bacc
import concourse.mybir as mybir
import concourse.tile as tile
from concourse.bass_utils import run_bass_kernel_spmd

BF16 = ml_dtypes.bfloat16
FP8E4 = ml_dtypes.float8_e4m3  # concourse float8e4 <-> ml_dtypes.float8_e4m3 (max 240)

V, D, NB = 32000, 768, 8
B, S = 64, 256
DT_STEP, STEPS_DESC, PI = 0.1, 5, 3

N_CORES = 8
BQ = B // 4          # batch rows per core (16)
KT = D // 128        # 6 k-tiles
MT = 3 * KT          # 18 m-tiles of gate rows
NT = BQ * S          # tokens per core (4096)
CHT = 512            # tokens per gather/input-matmul chunk
NCH = NT // CHT      # 8 chunks
BLK = 16             # recurrence steps per gi prefetch block
HEAD_CH = 2          # chunks produced before the step loop starts
IN_Q = 5             # interleaved input-phase quanta per step

# Whh quantization mode: 'fp8' (e4m3 + S scaling) or 'bf16'.  Measured:
# LDWEIGHTS is row-rate-limited (27ns for 128 rows regardless of dtype),
# so fp8 weights buy nothing -- default to bf16 for accuracy.
MODE = os.environ.get("KMODE", "bf16")
SCALE = 4096.0 if MODE == "fp8" else 1.0  # max |Whh|*S ~ 148 < 240 (e4m3 max)

# equal gate groups: A = chunks 0..2, B = chunks 3..5.  A's chain runs on
# DVE while B's burst occupies the PE; B's chain runs on GpSimd.
GROUPS = ((0, 3), (3, 6))

F32 = mybir.dt.float32
BF16_T = mybir.dt.bfloat16
FP8_T = mybir.dt.float8e4
I16 = mybir.dt.int16
AF = mybir.ActivationFunctionType

WHH_T = FP8_T if MODE == "fp8" else BF16_T

# gate-row permutation: m-tile j = (chunk c=j//3, gate g=j%3) covers rows
# g*768 + c*128 .. +128  ->  interleaved [r_c, z_c, n_c] blocks.
_PERM = np.concatenate(
    [np.arange(g * D + c * 128, g * D + (c + 1) * 128) for c in range(KT) for g in range(3)]
)


def _build_program(steps=S):
    nc = bacc.Bacc("TRN2", target_bir_lowering=False, debug=False, num_devices=N_CORES)

    t_ident = nc.dram_tensor("ident", [128, 128], BF16_T, kind="ExternalInput")
    t_bnst = nc.dram_tensor("bnst", [128, 256], BF16_T, kind="ExternalInput")
    t_ind = nc.dram_tensor("ind", [128, 3 * BQ], BF16_T, kind="ExternalInput")
    t_idx = nc.dram_tensor("idx", [128, NT // 16], I16, kind="ExternalInput")
    t_emb = nc.dram_tensor("emb", [V, D], BF16_T, kind="ExternalInput")
    t_wih = nc.dram_tensor("wihT", [128, KT * 3 * D], BF16_T, kind="ExternalInput")
    t_whh = nc.dram_tensor("whhT", [128, KT * 3 * D], WHH_T, kind="ExternalInput")
    t_bi = nc.dram_tensor("bias_i", [128, MT], F32, kind="ExternalInput")
    t_bn = nc.dram_tensor("bhhn", [128, KT * BQ], F32, kind="ExternalInput")
    t_out = nc.dram_tensor("h_out", [128, KT * BQ], F32, kind="ExternalOutput")

    W3D = 3 * D  # 2304
    n_blocks = steps // BLK
    n_chunks = max(1, (steps * BQ) // CHT)
    inv_s = 1.0 / SCALE

    with tile.TileContext(nc) as tc:
        with (
            tc.tile_pool(name="const", bufs=1) as const_pool,
            tc.tile_pool(name="dram", bufs=1, space="DRAM") as dram_pool,
            tc.tile_pool(name="xt", bufs=2) as xt_pool,
            tc.tile_pool(name="psum_in", bufs=3, space="PSUM") as psum_in,
            tc.tile_pool(name="gis", bufs=3) as gis_pool,
            tc.tile_pool(name="psA", bufs=2, space="PSUM") as psA_pool,
            tc.tile_pool(name="psB", bufs=2, space="PSUM") as psB_pool,
            tc.tile_pool(name="giblk", bufs=2) as giblk_pool,
            tc.tile_pool(name="hstate", bufs=1) as h_pool,
            tc.tile_pool(name="tmp", bufs=3) as tmp,
        ):
            ident_t = const_pool.tile([128, 128], BF16_T)
            nc.sync.dma_start(ident_t[:], t_ident.ap())
            bnst_t = const_pool.tile([128, 256], BF16_T)
            nc.sync.dma_start(bnst_t[:], t_bnst.ap())
            ind_t = const_pool.tile([128, 3 * BQ], BF16_T)
            nc.sync.dma_start(ind_t[:], t_ind.ap())
            idx_t = const_pool.tile([128, NT // 16], I16)
            wih_t = const_pool.tile([128, KT * W3D], BF16_T)
            whh_t = const_pool.tile([128, KT * W3D], WHH_T)
            bi_t = const_pool.tile([128, MT], F32)
            bn_t = const_pool.tile([128, KT * BQ], F32)
            nc.sync.dma_start(idx_t[:], t_idx.ap())
            nc.sync.dma_start(wih_t[:], t_wih.ap())
            nc.sync.dma_start(whh_t[:], t_whh.ap())
            nc.sync.dma_start(bi_t[:], t_bi.ap())
            nc.sync.dma_start(bn_t[:], t_bn.ap())

            giD = dram_pool.tile([MT, 128, NT], BF16_T)

            # ---- scheduler ordering helpers ----------------------------
            prev_pe_last = None   # last PE instr of previous segment

            def seg_edge(first_mm, last_mm):
                nonlocal prev_pe_last
                if first_mm is None:
                    return
                if prev_pe_last is not None:
                    tile.add_dep_helper(first_mm.ins, prev_pe_last.ins,
                                        sync=False, reason="pe segment order")
                prev_pe_last = last_mm

            act_prev = None

            def act_edge(op):
                nonlocal act_prev
                if act_prev is not None:
                    tile.add_dep_helper(op.ins, act_prev.ins, sync=False,
                                        reason="act order")
                act_prev = op

            # ---- input phase as a flat list of closures -----------------
            # each closure emits a tiny slice of (gather | matmul | act+store)
            # work and returns the matmul instruction if it emitted one.
            xt_tiles = {}
            ps_in_tiles = {}

            def mk_gather(nch):
                def f():
                    xt = xt_pool.tile([128, KT, CHT], BF16_T, name=f"xt{nch % 2}")
                    xt_tiles[nch] = xt
                    nc.gpsimd.dma_gather(
                        xt[:, :, :],
                        t_emb.ap(),
                        idx_t[:, nch * (CHT // 16):(nch + 1) * (CHT // 16)],
                        num_idxs=CHT,
                        num_idxs_reg=CHT,
                        elem_size=D,
                        transpose=True,
                    )
                    return None
                return f

            def mk_mm(nch, m, k):
                def f():
                    if k == 0:
                        ps_in_tiles[nch] = ps_in_tiles.get(nch, {})
                        ps_in_tiles[nch][m] = psum_in.tile([128, CHT], F32, name="ps_in")
                    mm = nc.tensor.matmul(
                        ps_in_tiles[nch][m][:],
                        wih_t[:, k * W3D + m * 128:k * W3D + (m + 1) * 128],
                        xt_tiles[nch][:, k, :],
                        start=(k == 0),
                        stop=(k == KT - 1),
                    )
                    return mm
                return f

            def mk_store(nch, m):
                def f():
                    gs = gis_pool.tile([128, CHT], BF16_T)
                    act = nc.scalar.activation(gs[:], ps_in_tiles[nch][m][:], AF.Identity,
                                               bias=bi_t[:, m:m + 1], scale=1.0)
                    act_edge(act)
                    nc.sync.dma_start(giD[m, :, nch * CHT:(nch + 1) * CHT], gs[:])
                    del ps_in_tiles[nch][m]
                    return None
                return f

            input_ops = []
            for nch in range(n_chunks):
                input_ops.append(mk_gather(nch))
                for m in range(MT):
                    for k in range(KT):
                        input_ops.append(mk_mm(nch, m, k))
                    input_ops.append(mk_store(nch, m))

            def pop_input(n):
                """Emit up to n input closures; return (first_mm, last_mm)."""
                first = last = None
                cnt = 0
                while input_ops and cnt < n:
                    mm = input_ops.pop(0)()
                    cnt += 1
                    if mm is not None:
                        if first is None:
                            first = mm
                        last = mm
                return first, last

            # ---- head: produce the first HEAD_CH chunks ----------------
            n_head = min(HEAD_CH, n_chunks)
            head_ops = n_head * (1 + MT * (KT + 1))
            f_h, l_h = pop_input(head_ops)
            seg_edge(f_h, l_h)

            # ---- persistent h state ------------------------------------
            # per-group ping-pong tiles; bf16 copy feeds the PE, fp32 is
            # the exact accumulator (gpsimd) and the final output.
            CA = GROUPS[0][1] - GROUPS[0][0]   # 4 chunks
            CB = GROUPS[1][1] - GROUPS[1][0]   # 2 chunks
            h_bA = [h_pool.tile([128, CA, BQ], BF16_T, name=f"hbA{i}", tag=f"hbA{i}") for i in range(2)]
            h_bB = [h_pool.tile([128, CB, BQ], BF16_T, name=f"hbB{i}", tag=f"hbB{i}") for i in range(2)]
            h_fA = [h_pool.tile([128, CA, BQ], F32, name=f"hfA{i}", tag=f"hfA{i}") for i in range(2)]
            h_fB = [h_pool.tile([128, CB, BQ], F32, name=f"hfB{i}", tag=f"hfB{i}") for i in range(2)]
            nc.vector.memset(h_bA[0][:], 0.0)
            nc.vector.memset(h_bB[0][:], 0.0)
            nc.gpsimd.memset(h_fA[0][:], 0.0)
            nc.gpsimd.memset(h_fB[0][:], 0.0)

            gi_blks = {}

            # gi_blk column layout per step: for each group [rz(2*nch) | n(nch)]
            # so the identity-matmul's moving operand and sigma's gi are
            # contiguous 2D slices.  slot(m): m=(c,g) -> group base + 2*cl+g
            # (rz) or base + 2*nch + cl (n).
            def gi_slot(m):
                c, g = m // 3, m % 3
                for (c0, c1) in GROUPS:
                    if c0 <= c < c1:
                        base, cl, nch = 3 * c0, c - c0, c1 - c0
                        return base + (2 * cl + g if g < 2 else 2 * nch + cl)
                raise AssertionError

            def load_block(b):
                gi = giblk_pool.tile([128, BLK, MT * BQ], BF16_T)
                gi_blks[b] = gi
                for m in range(MT):
                    sl = gi_slot(m)
                    nc.sync.dma_start(
                        gi[:, :, sl * BQ:(sl + 1) * BQ],
                        giD[m, :, b * BLK * BQ:(b + 1) * BLK * BQ].rearrange(
                            "p (t b) -> p t b", b=BQ
                        ),
                    )

            load_block(0)
            if n_blocks > 1:
                load_block(1)

            def h_src(group_idx, k, which):
                """moving-operand slice of h for contraction k-tile k."""
                if k < CA:
                    return (h_bA if which == "b" else h_fA)[group_idx][:, k, :]
                return (h_bB if which == "b" else h_fB)[group_idx][:, k - CA, :]

            # ---- recurrence --------------------------------------------
            # Per step, PE order: [input quanta][A-burst][B-burst], bursts
            # k-outermost so next-step matmuls unblock per h chunk-group.
            # Both gate chains run on DVE+ACT with baseline's proven
            # interleaved ordering; gpsimd takes zh and the fp32 state.
            for t in range(steps):
                cur, nxt = t % 2, (t + 1) % 2
                blk, tl = t // BLK, t % BLK

                if t % BLK == 0 and blk + 2 < n_blocks:
                    load_block(blk + 2)
                if t % BLK == 0 and blk - 2 in gi_blks:
                    del gi_blks[blk - 2]
                gi_blk = gi_blks[blk]

                f_in, l_in = pop_input(IN_Q)
                seg_edge(f_in, l_in)

                ops = {}
                for gidx, (c0, c1) in enumerate(GROUPS):
                    nch_g = c1 - c0
                    pool_g = psA_pool if gidx == 0 else psB_pool
                    nrz = 2 * nch_g * BQ
                    ps = pool_g.tile([128, 3 * nch_g * BQ], F32, name=f"ps{gidx}")
                    ops[f"ps{gidx}"] = ps
                    ops[f"girz{gidx}"] = gi_blk[:, tl, 3 * c0 * BQ:3 * c0 * BQ + nrz]
                    ops[f"gin{gidx}"] = gi_blk[
                        :, tl, 3 * c0 * BQ + nrz:3 * c1 * BQ]
                    ops[f"bn{gidx}"] = bn_t[:, c0 * BQ:c1 * BQ]

                    # burst: the identity-mm is the psum group's single
                    # start (start=True resets has_written for the whole
                    # bank, so exactly one start per tile is legal); it
                    # seeds the rz block with gi_rz.  The recurrent matmuls
                    # run k-outermost (start=False: first write to each
                    # element overwrites, later ones accumulate) so
                    # next-step matmuls unblock as each h chunk lands.
                    first = nc.tensor.matmul(
                        ps[:, 0:nrz],
                        ident_t[:],
                        ops[f"girz{gidx}"],
                        start=True, stop=False,
                        skip_group_check=True,
                    )
                    # bn-fold: one matmul adds bhh_n into the n block
                    # (stationary row j = bn of chunk c0+j, moving =
                    # indicator).  start=False writes commute via
                    # has_written, so no ordering constraint vs the k-mms.
                    bnmm = nc.tensor.matmul(
                        ps[:, nrz:],
                        bnst_t[:, gidx * 128:(gidx + 1) * 128],
                        ind_t[:],
                        start=False, stop=False,
                        skip_group_check=True,
                    )
                    tile.add_dep_helper(bnmm.ins, first.ins, sync=False,
                                        reason="group start first")
                    last = bnmm
                    n_mm = 3 * nch_g * KT
                    cnt = 0
                    for k in range(KT):
                        for ci in range(nch_g):
                            for g in range(3):
                                m = (c0 + ci) * 3 + g
                                sl = (2 * ci + g if g < 2 else 2 * nch_g + ci)
                                cnt += 1
                                mm = nc.tensor.matmul(
                                    ps[:, sl * BQ:(sl + 1) * BQ],
                                    whh_t[:, k * W3D + m * 128:k * W3D + (m + 1) * 128],
                                    h_src(cur, k, "b"),
                                    start=False,
                                    stop=(cnt == n_mm),
                                    skip_group_check=True,
                                )
                                tile.add_dep_helper(
                                    mm.ins, first.ins, sync=False,
                                    reason="group start first")
                                last = mm
                    seg_edge(first, last)

                for gidx, (c0, c1) in enumerate(GROUPS):
                    nch_g = c1 - c0
                    nrz = 2 * nch_g * BQ
                    ps = ops[f"ps{gidx}"]
                    h_b_n = (h_bA if gidx == 0 else h_bB)[nxt]
                    h_f_c = (h_fA if gidx == 0 else h_fB)[cur]
                    h_f_n = (h_fA if gidx == 0 else h_fB)[nxt]

                    rz = tmp.tile([128, nrz], F32, tag=f"rz{gidx}")
                    ops[f"sig{gidx}"] = nc.scalar.activation(
                        rz[:], ps[:, 0:nrz], AF.Sigmoid, scale=inv_s)
                    rz4 = rz[:].rearrange("p (c g b) -> p c g b", g=2, b=BQ)
                    r_s, z_s = rz4[:, :, 0, :], rz4[:, :, 1, :]
                    zh = tmp.tile([128, nch_g, BQ], F32, tag=f"zh{gidx}")
                    nc.gpsimd.tensor_mul(zh[:], z_s, h_f_c[:])
                    u = tmp.tile([128, nch_g, BQ], F32, tag=f"u{gidx}")
                    ops[f"u{gidx}"] = nc.vector.tensor_mul(
                        u[:], r_s,
                        ps[:, nrz:].rearrange("p (c b) -> p c b", b=BQ))
                    v = tmp.tile([128, nch_g, BQ], F32, tag=f"v{gidx}")
                    ops[f"v{gidx}"] = nc.vector.tensor_add(
                        v[:], u[:],
                        ops[f"gin{gidx}"].rearrange("p (c b) -> p c b", b=BQ))
                    n_t = tmp.tile([128, nch_g, BQ], F32, tag=f"n{gidx}")
                    ops[f"tanh{gidx}"] = nc.scalar.activation(
                        n_t[:], v[:], AF.Tanh, scale=inv_s)
                    q = tmp.tile([128, nch_g, BQ], F32, tag=f"q{gidx}")
                    ops[f"q{gidx}"] = nc.vector.tensor_scalar(
                        q[:], z_s, -1.0, 1.0,
                        mybir.AluOpType.mult, mybir.AluOpType.add)
                    w_t = tmp.tile([128, nch_g, BQ], F32, tag=f"w{gidx}")
                    ops[f"w{gidx}"] = nc.vector.tensor_mul(w_t[:], q[:], n_t[:])
                    ops[f"hb{gidx}"] = nc.vector.tensor_add(
                        h_b_n[:], w_t[:], zh[:])
                    nc.gpsimd.tensor_add(h_f_n[:], w_t[:], zh[:])

                # enforce the engine orders that keep both chains tight:
                # finish A's chain first (it feeds the next burst's first
                # matmuls), then B's.
                dve_order = ["u0", "v0", "q0", "w0", "hb0",
                             "u1", "v1", "q1", "w1", "hb1"]
                for a, b in zip(dve_order, dve_order[1:]):
                    tile.add_dep_helper(ops[b].ins, ops[a].ins, sync=False,
                                        reason="dve chain order")
                act_edge(ops["sig0"])
                act_edge(ops["tanh0"])
                act_edge(ops["sig1"])
                act_edge(ops["tanh1"])

            # drain any remaining input work (only for tiny debug step counts)
            while input_ops:
                pop_input(len(input_ops))

            fin = steps % 2
            nc.sync.dma_start(
                t_out.ap().rearrange("p (c b) -> p c b", b=BQ)[:, 0:CA, :],
                h_fA[fin][:])
            nc.sync.dma_start(
                t_out.ap().rearrange("p (c b) -> p c b", b=BQ)[:, CA:KT, :],
                h_fB[fin][:])

    nc.compile()
    return nc


def _pack_encoder(emb, Wih, Whh, bih, bhh):
    """Host-side prep of one encoder's parameters into device layouts."""
    emb_bf = np.ascontiguousarray(emb.astype(BF16))
    Wp = (Wih[_PERM] * SCALE)  # [2304, 768], pre-scaled
    wihT = np.ascontiguousarray(
        Wp.reshape(3 * D, KT, 128).transpose(2, 1, 0).reshape(128, KT * 3 * D).astype(BF16)
    )
    Wp = Whh[_PERM] * SCALE
    whhT_f = Wp.reshape(3 * D, KT, 128).transpose(2, 1, 0).reshape(128, KT * 3 * D)
    if MODE == "fp8":
        whhT = np.ascontiguousarray(whhT_f.astype(FP8E4))
    else:
        whhT = np.ascontiguousarray(whhT_f.astype(BF16))
    bias_vec = (bih + np.concatenate([bhh[:D], bhh[D:2 * D], np.zeros(D, np.float32)]))[_PERM] * SCALE
    bias_i = np.ascontiguousarray(bias_vec.reshape(MT, 128).T.astype(np.float32))
    bhh_n = bhh[2 * D:] * SCALE
    bhhn = np.ascontiguousarray(
        np.repeat(bhh_n.reshape(KT, 128).T[:, :, None], BQ, axis=2).reshape(128, KT * BQ).astype(np.float32)
    )
    # bn-fold stationary: per group, row j = bhh_n of chunk c0+j
    bnst = np.zeros((128, 256), np.float32)
    for gidx, c0 in enumerate((0, 3)):
        for j in range(3):
            bnst[j, gidx * 128:(gidx + 1) * 128] = bhh_n[(c0 + j) * 128:(c0 + j + 1) * 128]
    return emb_bf, wihT, whhT, bias_i, bhhn, np.ascontiguousarray(bnst.astype(BF16))


_CACHE = {}


def run_device(inputs, steps=S, trace=False):
    """Run the 8-core device program; returns (h_ctx [64,768], h_tgt [64,768], perf)."""
    key = (steps, MODE)
    if key not in _CACHE:
        _CACHE[key] = _build_program(steps)
    nc = _CACHE[key]

    ctx_tok = np.asarray(inputs["ctx"]).astype(np.int16)      # [64, 256]
    tgt_tok = np.asarray(inputs["tgt_seq"]).astype(np.int16)  # [64, 256]

    enc_ctx = _pack_encoder(
        np.asarray(inputs["emb"], np.float32), np.asarray(inputs["Wih"], np.float32),
        np.asarray(inputs["Whh"], np.float32), np.asarray(inputs["bih"], np.float32),
        np.asarray(inputs["bhh"], np.float32),
    )
    enc_tgt = _pack_encoder(
        np.asarray(inputs["t_emb"], np.float32), np.asarray(inputs["t_Wih"], np.float32),
        np.asarray(inputs["t_Whh"], np.float32), np.asarray(inputs["t_bih"], np.float32),
        np.asarray(inputs["t_bhh"], np.float32),
    )

    ind = np.zeros((128, 3 * BQ), np.float32)
    for c in range(3):
        ind[c, c * BQ:(c + 1) * BQ] = 1.0
    ind = np.ascontiguousarray(ind.astype(BF16))

    in_maps = []
    for core in range(N_CORES):
        e, q = core // 4, core % 4
        emb_bf, wihT, whhT, bias_i, bhhn, bnst = enc_ctx if e == 0 else enc_tgt
        toks = (ctx_tok if e == 0 else tgt_tok)[q * BQ:(q + 1) * BQ, :]  # [16, 256]
        # gather position i = t*16+b reads idx[i%16, i//16] = toks[b, t]; the
        # [16, NT/16] block must be replicated into each gpsimd core's stripe.
        idx = np.tile(toks, (8, 1))
        in_maps.append({
            "ident": np.eye(128, dtype=np.float32).astype(BF16),
            "bnst": bnst,
            "ind": ind,
            "idx": idx,
            "emb": emb_bf,
            "wihT": wihT,
            "whhT": whhT,
            "bias_i": bias_i,
            "bhhn": bhhn,
        })

    res = run_bass_kernel_spmd(nc, in_maps, core_ids=list(range(N_CORES)), trace=trace)

    def unpack_h(outs):
        # out [128, KT*BQ]: out[p, k*BQ + b] = h[b, k*128 + p]
        h = np.zeros((4 * BQ, D), np.float32)
        for q in range(4):
            o = np.asarray(outs[q]["h_out"], np.float32).reshape(128, KT, BQ)
            h[q * BQ:(q + 1) * BQ, :] = o.transpose(2, 1, 0).reshape(BQ, D)
        return h

    h_ctx = unpack_h(res.results[0:4])
    h_tgt = unpack_h(res.results[4:8])
    return h_ctx, h_tgt, res


def _head(h_ctx, h_tgt, inputs):
    """Final tiny math on host, float64 for exactness."""
    Wfc = np.asarray(inputs["Wfc"], np.float64)
    bfc = np.asarray(inputs["bfc"], np.float64)
    tWfc = np.asarray(inputs["t_Wfc"], np.float64)
    tbfc = np.asarray(inputs["t_bfc"], np.float64)
    We = np.asarray(inputs["We"], np.float64)
    u0 = np.asarray(inputs["u_sn"], np.float64)

    ctx_latent = h_ctx.astype(np.float64) @ Wfc.T + bfc          # [64, 8]
    target_latent = h_tgt.astype(np.float64) @ tWfc.T + tbfc     # [64, 8]

    u = u0 / (np.linalg.norm(u0) + 1e-12)
    for _ in range(PI):
        v = We.T @ u
        v = v / (np.linalg.norm(v) + 1e-12)
        u = We @ v
        u = u / (np.linalg.norm(u) + 1e-12)
    sigma = u @ (We @ v)
    Wsn = We / sigma

    pred_latent = -(STEPS_DESC * DT_STEP) * (ctx_latent @ Wsn.T)  # [64, 8]
    return (
        pred_latent.astype(np.float32)[:, None, :],
        target_latent.astype(np.float32)[:, None, :],
    )


def kernel(**inputs):
    h_ctx, h_tgt, _ = run_device(inputs, steps=S, trace=False)
    return _head(h_ctx, h_tgt, inputs)
